# revision 19
# baseline (speedup 1.0000x reference)
"""Multi-head attention (B=4, S=2048, H=16, D=64, C=1024) on 8 NeuronCores.

Sharding: core c handles batch b=c//2 and head-half half=c%2 (8 heads = 512
inner dims).  Each core computes q/k/v projections for its half of the heads,
full softmax attention over S=2048, and a partial output projection through
its 512 rows of Wo.  Host sums the two partials per batch and adds the bias.

Per-core kernel layout (all matmul operands bf16, PSUM accumulation fp32):
  xt    [C=1024, S=2048]   hidden_states[b].T          (host pre-transposed)
  wq/wk/wv [C, I=512]      per-half weight columns
  wo    [I=512, C=1024]    per-half weight rows
  qT,kT [I, S] stored as 4 SBUF tiles [128, 2048]  (head pair per tile)
  v_pad [S, 8*65]          v with a ones column per head (row-sum via matmul)
  scores^T per (pair, qi-chunk, kj-tile): [kj=128, qi=512] via row-tiled
  (K=64) matmul pairs; exp on ScalarE; p@[v|1] accumulated in PSUM over kj.
"""

import functools

import numpy as np
import ml_dtypes

S = 2048          # sequence length
C = 1024          # query dim
I = 512           # inner dims per core (8 heads x 64)
HC = 8            # heads per core
D = 64            # head dim
NCORES = 8
SCALE = D ** -0.5
CT = C // 128     # 8 c-tiles
IT = I // 128     # 4 i-tiles (head pairs)
ST = S // 128     # 16 s-tiles
NQ = S // 512     # 4 qi chunks
VW = D + 1        # 65: v plus ones column


def _build(repeat=1, phases="dma,proj,attn,outproj", fused_exp=True):
    import contextlib

    import concourse.bacc as bacc
    import concourse.tile as tile
    from concourse import mybir

    f32 = mybir.dt.float32
    bf16 = mybir.dt.bfloat16
    Exp = mybir.ActivationFunctionType.Exp

    nc = bacc.Bacc("TRN2", target_bir_lowering=False, debug=False,
                   num_devices=NCORES)

    # All inputs are host-pre-shuffled to partition-major [128, ...] layouts so
    # every DMA reads long contiguous runs per partition.
    xt_d = nc.dram_tensor("xt", [128, CT * S], bf16, kind="ExternalInput").ap()
    wq_d = nc.dram_tensor("wq", [128, CT * I], bf16, kind="ExternalInput").ap()
    wk_d = nc.dram_tensor("wk", [128, CT * I], bf16, kind="ExternalInput").ap()
    wv_d = nc.dram_tensor("wv", [128, CT * I], bf16, kind="ExternalInput").ap()
    wo_d = nc.dram_tensor("wo", [128, IT * C], bf16, kind="ExternalInput").ap()
    out_d = nc.dram_tensor("out", [S, C], bf16, kind="ExternalOutput").ap()

    with tile.TileContext(nc) as tc:
        with contextlib.ExitStack() as ctx:
            const = ctx.enter_context(tc.tile_pool(name="const", bufs=1))
            # ---- load inputs ONCE, before the timing loop -------------------
            # (loop-invariant; the loop body only reads them).  Three DMA
            # issue paths: sync + scalar HWDGE queues, Pool SWDGE.
            xt_sb = const.tile([128, CT, S], bf16)
            xt_r = xt_d.rearrange("p (t s) -> p t s", s=S)
            wq_sb = const.tile([128, CT, I], bf16)
            nc.scalar.dma_start(out=wq_sb,
                                in_=wq_d.rearrange("p (t i) -> p t i", i=I))
            for c in range(0, CT, 2):
                nc.sync.dma_start(out=xt_sb[:, c:c + 1, :],
                                  in_=xt_r[:, c:c + 1, :])
                nc.gpsimd.dma_start(out=xt_sb[:, c + 1:c + 2, :],
                                    in_=xt_r[:, c + 1:c + 2, :])
            wk_sb = const.tile([128, CT, I], bf16)
            nc.scalar.dma_start(out=wk_sb,
                                in_=wk_d.rearrange("p (t i) -> p t i", i=I))
            wv_sb = const.tile([128, CT, I], bf16)
            nc.scalar.dma_start(out=wv_sb,
                                in_=wv_d.rearrange("p (t i) -> p t i", i=I))
            wo_sb = const.tile([128, IT, C], bf16)
            nc.sync.dma_start(out=wo_sb,
                              in_=wo_d.rearrange("p (t c) -> p t c", c=C))
            ones_sb = const.tile([1, 128], bf16)
            nc.vector.memset(ones_sb, 1.0)

            iv = None
            if repeat > 1:
                # staggered_reset: no all-engine barrier / reset block on the
                # back-edge — engines flow into the next iteration as data
                # deps allow.
                iv = ctx.enter_context(
                    tc.For_i(0, repeat, 1, staggered_reset=True))
            work = ctx.enter_context(tc.tile_pool(name="work", bufs=4))
            outp = ctx.enter_context(tc.tile_pool(name="outp", bufs=4))
            ps_pool = ctx.enter_context(tc.tile_pool(name="ps", bufs=3, space="PSUM"))
            pv_pool = ctx.enter_context(tc.tile_pool(name="pv", bufs=2, space="PSUM"))

            phs = set(phases.split(","))

            # ---- microbenchmarks (phases="dma,bench_<kind>_<n>[_<cols>]") --
            bench = [t for t in phs if t.startswith("bench_")]
            if bench:
                parts = bench[0].split("_")
                kind = parts[1]
                if kind == "mm":
                    n, cols = int(parts[2]), int(parts[3])
                    for i in range(n):
                        acc = ps_pool.tile([128, cols], f32, tag="ps",
                                           name="mmb")
                        nc.tensor.matmul(
                            acc, lhsT=wq_sb[:, i % CT, 0:128],
                            rhs=xt_sb[:, i % CT, 0:cols],
                            start=True, stop=True)
                elif kind in ("act", "actns"):
                    n = int(parts[2])
                    acc0 = ps_pool.tile([128, 1024], f32, tag="ps",
                                        name="actsrc")
                    nc.vector.memset(acc0, 1.0)
                    for i in range(n):
                        pb = work.tile([128, 1024], bf16, tag="p", bufs=8,
                                       name="pb")
                        if kind == "act":
                            nc.scalar.activation(out=pb, in_=acc0, func=Exp,
                                                 scale=SCALE)
                        else:
                            nc.scalar.activation(out=pb, in_=acc0, func=Exp)
                elif kind == "pair":
                    # two concurrent-candidate K=64 row-tiled matmuls per
                    # iteration (the attention scores pattern)
                    n = int(parts[2])
                    for i in range(n):
                        acc = ps_pool.tile([128, 1024], f32, tag="ps",
                                           name="prb")
                        nc.tensor.matmul(
                            acc[:, 0:512], lhsT=wq_sb[0:64, i % CT, 0:128],
                            rhs=xt_sb[0:64, i % CT, 0:512],
                            start=True, stop=True, tile_position=(0, 0))
                        nc.tensor.matmul(
                            acc[:, 512:1024],
                            lhsT=wq_sb[64:128, i % CT, 0:128],
                            rhs=xt_sb[64:128, i % CT, 0:512],
                            start=True, stop=True, tile_position=(64, 0))
                elif kind == "pv":
                    # PV-style matmul: lhsT only 65 columns
                    n = int(parts[2])
                    for i in range(n):
                        acc = ps_pool.tile([VW, 512], f32, tag="pvb", bufs=4,
                                           name="pvb")
                        nc.tensor.matmul(
                            acc, lhsT=xt_sb[:, i % CT, 0:VW],
                            rhs=wq_sb[:, i % CT, 0:512],
                            start=True, stop=True)
                elif kind == "mmw":
                    # same stationary weight every time — does the weight
                    # reload disappear?
                    n, cols = int(parts[2]), int(parts[3])
                    for i in range(n):
                        acc = ps_pool.tile([128, cols], f32, tag="ps",
                                           name="mmwb")
                        nc.tensor.matmul(
                            acc, lhsT=wq_sb[:, 0, 0:128],
                            rhs=xt_sb[:, 0, 0:cols],
                            start=True, stop=True)
                elif kind == "chain":
                    # mimics one attention kt step: scores pair -> exp ->
                    # PV pair, rotating psum tiles
                    n = int(parts[2])
                    qT0 = const.tile([128, 512], bf16)
                    kT0 = const.tile([128, 128], bf16)
                    v0 = const.tile([128, 2 * VW], bf16)
                    nc.vector.memset(qT0, 0.001)
                    nc.vector.memset(kT0, 0.001)
                    nc.vector.memset(v0, 1.0)
                    for i in range(n):
                        acc = ps_pool.tile([128, 1024], f32, tag="ps",
                                           name="cb")
                        nc.tensor.matmul(
                            acc[:, 0:512], lhsT=kT0[0:64, :],
                            rhs=qT0[0:64, :],
                            start=True, stop=True, tile_position=(0, 0))
                        nc.tensor.matmul(
                            acc[:, 512:1024], lhsT=kT0[64:128, :],
                            rhs=qT0[64:128, :],
                            start=True, stop=True, tile_position=(64, 0))
                        pc = work.tile([128, 1024], bf16, tag="p", bufs=8,
                                       name="pc")
                        nc.scalar.activation(out=pc, in_=acc, func=Exp,
                                             scale=SCALE)
                        oA = pv_pool.tile([VW, 512], f32, tag="pv", name="cA")
                        nc.tensor.matmul(oA, lhsT=v0[:, 0:VW],
                                         rhs=pc[:, 0:512],
                                         start=True, stop=True)
                        oB = pv_pool.tile([VW, 512], f32, tag="pv", name="cB")
                        nc.tensor.matmul(oB, lhsT=v0[:, VW:2 * VW],
                                         rhs=pc[:, 512:1024],
                                         start=True, stop=True)

            # ---- projections -------------------------------------------------
            qT_sb = const.tile([128, IT, S], bf16)
            kT_sb = const.tile([128, IT, S], bf16)
            v_sb = const.tile([128, ST, HC * VW], bf16)
            # ones columns per head (softmax denominator): contiguous memset of
            # the whole tile — the v copies then overwrite the 64 data columns.
            # (A strided single-element memset here measures ~78us on HW.)
            nc.vector.memset(v_sb, 1.0)

            oT_sb = const.tile([128, IT, S], bf16)

            # ---- projections as weavable micro-steps ------------------------
            # each step emits ONE PE matmul; the last step of an accumulation
            # also emits the PSUM evacuation copy (Pool engine).  Steps are
            # either run back-to-back (startup) or popped a few per attention
            # kt-step so they hide under the ACT-bound exp stream.
            if "proj" in phs:
                def proj_qk_steps(it, which, nq):
                    w_sb, o_sb = ((wq_sb, qT_sb), (wk_sb, kT_sb))[which]
                    st8 = {}

                    def mk(ct, h2):
                        def f():
                            if ct == 0 and h2 == 0:
                                st8["acc"] = ps_pool.tile(
                                    [128, 1024], f32, tag="pj", bufs=1,
                                    name="pj_ps")
                            nc.tensor.matmul(
                                st8["acc"][:, h2 * 512:(h2 + 1) * 512],
                                lhsT=w_sb[:, ct, it * 128:(it + 1) * 128],
                                rhs=xt_sb[:, ct, nq * 1024 + h2 * 512:
                                          nq * 1024 + (h2 + 1) * 512],
                                start=(ct == 0), stop=(ct == CT - 1))
                            if ct == CT - 1 and h2 == 1:
                                nc.vector.tensor_copy(
                                    out=o_sb[:, it, nq * 1024:(nq + 1) * 1024],
                                    in_=st8["acc"])
                        return f
                    return [mk(ct, h2) for ct in range(CT) for h2 in range(2)]

                v_main = v_sb.rearrange("p t (h e) -> p t h e", e=VW)[:, :, :, 0:D]

                def proj_v_steps(st):
                    st8 = {}

                    def mk(ct):
                        def f():
                            if ct == 0:
                                st8["acc"] = ps_pool.tile(
                                    [128, 512], f32, tag="pj", bufs=1,
                                    name="v_ps")
                            nc.tensor.matmul(
                                st8["acc"],
                                lhsT=xt_sb[:, ct, st * 128:(st + 1) * 128],
                                rhs=wv_sb[:, ct, :],
                                start=(ct == 0), stop=(ct == CT - 1))
                            if ct == CT - 1:
                                nc.vector.tensor_copy(
                                    out=v_main[:, st],
                                    in_=st8["acc"].rearrange(
                                        "p (h d) -> p h d", d=D))
                        return f
                    return [mk(ct) for ct in range(CT)]

            # ---- output projection as weavable micro-steps ------------------
            def outproj_steps(st):
                st8 = {}

                def mk(it, h2):
                    def f():
                        if it == 0 and h2 == 0:
                            st8["acc"] = ps_pool.tile(
                                [128, 1024], f32, tag="pj", bufs=1,
                                name="out_ps")
                        nc.tensor.matmul(
                            st8["acc"][:, h2 * 512:(h2 + 1) * 512],
                            lhsT=oT_sb[:, it, st * 128:(st + 1) * 128],
                            rhs=wo_sb[:, it, h2 * 512:(h2 + 1) * 512],
                            start=(it == 0), stop=(it == IT - 1))
                        if it == IT - 1 and h2 == 1:
                            ob = outp.tile([128, 1024], bf16, tag="ob",
                                           name="ob")
                            nc.vector.tensor_copy(out=ob, in_=st8["acc"])
                            eng = nc.sync if st % 2 == 0 else nc.gpsimd
                            eng.dma_start(
                                out=out_d[st * 128:(st + 1) * 128, :], in_=ob)
                    return f
                return [mk(it, h2) for it in range(IT) for h2 in range(2)]

            # ---- attention ---------------------------------------------------
            from collections import deque
            weave = deque()

            def run_steps(steps):
                for f in steps:
                    f()

            def attn_chunk(hp, nq, budget=2):
                hA, hB = 2 * hp, 2 * hp + 1
                qs = slice(nq * 512, (nq + 1) * 512)
                oA = pv_pool.tile([VW, 512], f32, tag="pv", bufs=2, name="oA")
                oB = pv_pool.tile([VW, 512], f32, tag="pv", bufs=2, name="oB")

                def emit_scores(kt):
                    ks = slice(kt * 128, (kt + 1) * 128)
                    sAB = ps_pool.tile([128, 1024], f32, tag="ps", bufs=2,
                                       name="sAB")
                    nc.tensor.matmul(
                        sAB[:, 0:512],
                        lhsT=kT_sb[0:64, hp, ks], rhs=qT_sb[0:64, hp, qs],
                        start=True, stop=True, tile_position=(0, 0))
                    nc.tensor.matmul(
                        sAB[:, 512:1024],
                        lhsT=kT_sb[64:128, hp, ks], rhs=qT_sb[64:128, hp, qs],
                        start=True, stop=True, tile_position=(64, 0))
                    pAB = work.tile([128, 1024], bf16, tag="p", bufs=8,
                                    name="pAB")
                    # SCALE is folded into wk host-side (exact: x 2^-3)
                    nc.scalar.activation(out=pAB, in_=sAB, func=Exp)
                    return pAB

                def emit_pv(kt, pAB):
                    nc.tensor.matmul(
                        oA, lhsT=v_sb[:, kt, hA * VW:(hA + 1) * VW],
                        rhs=pAB[:, 0:512],
                        start=(kt == 0), stop=(kt == ST - 1))
                    nc.tensor.matmul(
                        oB, lhsT=v_sb[:, kt, hB * VW:(hB + 1) * VW],
                        rhs=pAB[:, 512:1024],
                        start=(kt == 0), stop=(kt == ST - 1))

                # software pipeline: PV for tile kt trails the scores+exp of
                # kt+1 so the PE never sits behind the ACT exp latency, and up
                # to `budget` woven projection matmuls ride in the ACT slack.
                pend = None
                for kt in range(ST):
                    w = budget
                    while w and weave:
                        weave.popleft()()
                        w -= 1
                    pAB = emit_scores(kt)
                    if pend is not None:
                        emit_pv(*pend)
                    pend = (kt, pAB)
                emit_pv(*pend)
                # stage raw accumulators to SBUF on Pool (frees the pv psum
                # ring fast), then normalize: recip on DVE, partition
                # broadcast + multiply on Pool.
                oS = work.tile([VW, 1024], f32, tag="oS", bufs=2, name="oS")
                nc.vector.tensor_copy(out=oS[:, 0:512], in_=oA)
                nc.vector.tensor_copy(out=oS[:, 512:1024], in_=oB)
                rAB = work.tile([1, 1024], f32, tag="recip", bufs=2,
                                name="rAB")
                nc.vector.reciprocal(out=rAB, in_=oS[D:VW, :])
                bcAB = work.tile([64, 1024], f32, tag="bcs", bufs=2,
                                 name="bcAB")
                nc.gpsimd.partition_broadcast(bcAB, rAB)
                nc.vector.tensor_mul(
                    out=oT_sb[0:64, hp, qs], in0=oS[0:D, 0:512],
                    in1=bcAB[:, 0:512])
                nc.vector.tensor_mul(
                    out=oT_sb[64:128, hp, qs], in0=oS[0:D, 512:1024],
                    in1=bcAB[:, 512:1024])

            if "attn" in phs:
                # startup: q/k projections for pair 0 and the first 4 v tiles;
                # everything else weaves into attention kt-steps.
                if "proj" in phs:
                    for which in range(2):
                        for nq2 in range(2):
                            run_steps(proj_qk_steps(0, which, nq2))
                    for st in range(4):
                        run_steps(proj_v_steps(st))
                    for st in range(4, ST):
                        weave.extend(proj_v_steps(st))
                for hp in range(IT):
                    if "proj" in phs and hp < IT - 1:
                        for which in range(2):
                            for nq2 in range(2):
                                weave.extend(proj_qk_steps(hp + 1, which, nq2))
                    for nq in range(NQ):
                        if hp == IT - 1 and "outproj" in phs and nq >= 1:
                            for st in range(4 * (nq - 1), 4 * nq):
                                weave.extend(outproj_steps(st))
                        # ACT slack fits ~1.25 woven matmuls per kt; hp 1-2
                        # weave exactly 64 qk steps over 64 slots.  hp 0 and
                        # 3 carry more (v tail + startup spill / outproj).
                        attn_chunk(hp, nq,
                                   budget=7 if (hp, nq) == (0, 0)
                                   else (2 if hp in (0, IT - 1) else 1))
                while weave:
                    weave.popleft()()
                if "outproj" in phs:
                    for st in range(12, 16):
                        run_steps(outproj_steps(st))
            else:
                if "proj" in phs:
                    for it in range(IT):
                        for which in range(2):
                            for nq2 in range(2):
                                run_steps(proj_qk_steps(it, which, nq2))
                    for st in range(ST):
                        run_steps(proj_v_steps(st))
                if "outproj" in phs:
                    for st in range(ST):
                        run_steps(outproj_steps(st))

    nc.compile()
    return nc


@functools.lru_cache(maxsize=8)
def _built(repeat=1, phases="dma,proj,attn,outproj", fused_exp=True):
    return _build(repeat, phases, fused_exp)


def _pm(a):
    """[T*128, F] -> partition-major [128, T*F] (bf16)."""
    T = a.shape[0] // 128
    return np.ascontiguousarray(
        a.reshape(T, 128, a.shape[1]).swapaxes(0, 1).reshape(128, -1)
    ).astype(ml_dtypes.bfloat16)


def _in_maps(hidden_states, Wq, Wk, Wv, Wo):
    maps = []
    for c in range(NCORES):
        b, half = divmod(c, 2)
        sl = slice(half * I, (half + 1) * I)
        maps.append({
            "xt": _pm(np.ascontiguousarray(hidden_states[b].T)),
            "wq": _pm(Wq[:, sl]),
            # softmax scale folded in here: x 2^-3, exact in floating point
            "wk": _pm(Wk[:, sl] * SCALE),
            "wv": _pm(Wv[:, sl]),
            "wo": _pm(Wo[sl, :]),
        })
    return maps


@functools.lru_cache(maxsize=1)
def _runner():
    """Compile the SPMD program once and return a function
    maps -> list of per-core output dicts."""
    import jax
    from jax.sharding import Mesh, PartitionSpec, NamedSharding
    from jax.experimental.shard_map import shard_map

    import concourse.mybir as mybir
    from concourse.bass2jax import (
        _bass_exec_p, install_neuronx_cc_hook, partition_id_tensor)

    nc = _built()
    install_neuronx_cc_hook()
    partition_name = nc.partition_id_tensor.name if nc.partition_id_tensor else None

    in_names, out_names, out_avals, zero_outs = [], [], [], []
    for alloc in nc.m.functions[0].allocations:
        if not isinstance(alloc, mybir.MemoryLocationSet):
            continue
        name = alloc.memorylocations[0].name
        if alloc.kind == "ExternalInput":
            if name != partition_name:
                in_names.append(name)
        elif alloc.kind == "ExternalOutput":
            out_names.append(name)
            shape = tuple(alloc.tensor_shape)
            dtype = mybir.dt.np(alloc.dtype)
            out_avals.append(jax.core.ShapedArray(shape, dtype))
            zero_outs.append(np.zeros(shape, dtype))
    n_params = len(in_names)
    all_in_names = in_names + out_names
    if partition_name is not None:
        all_in_names = all_in_names + [partition_name]

    def _body(*args):
        operands = list(args)
        if partition_name is not None:
            operands.append(partition_id_tensor())
        return tuple(_bass_exec_p.bind(
            *operands,
            out_avals=tuple(out_avals),
            in_names=tuple(all_in_names),
            out_names=tuple(out_names),
            lowering_input_output_aliases=(),
            sim_require_finite=True,
            sim_require_nnan=True,
            nc=nc,
        ))

    devices = jax.devices()[:NCORES]
    mesh = Mesh(np.asarray(devices), ("core",))
    in_specs = (PartitionSpec("core"),) * (n_params + len(out_names))
    out_specs = (PartitionSpec("core"),) * len(out_names)
    sharded = jax.jit(
        shard_map(_body, mesh=mesh, in_specs=in_specs, out_specs=out_specs,
                  check_rep=False),
        keep_unused=True,
    )
    sharding = NamedSharding(mesh, PartitionSpec("core"))
    dev_zero = [jax.device_put(
        np.zeros((NCORES * z.shape[0], *z.shape[1:]), z.dtype), sharding)
        for z in zero_outs]

    def run(maps):
        concat_in = [np.concatenate([np.asarray(maps[c][n]) for c in range(NCORES)],
                                    axis=0) for n in in_names]
        dev_in = [jax.device_put(a, sharding) for a in concat_in]
        out_arrs = sharded(*dev_in, *dev_zero)
        return [
            {n: np.asarray(out_arrs[i]).reshape(NCORES, *out_avals[i].shape)[c]
             for i, n in enumerate(out_names)}
            for c in range(NCORES)
        ]

    return run


def kernel(hidden_states, Wq, Wk, Wv, Wo, bo):
    maps = _in_maps(np.asarray(hidden_states), np.asarray(Wq), np.asarray(Wk),
                    np.asarray(Wv), np.asarray(Wo))
    results = _runner()(maps)
    B = hidden_states.shape[0]
    out = np.empty((B, S, C), np.float32)
    for b in range(B):
        out[b] = (results[2 * b]["out"].astype(np.float32)
                  + results[2 * b + 1]["out"].astype(np.float32))
    out += np.asarray(bo, np.float32)
    return out

